# revision 19
# baseline (speedup 1.0000x reference)
"""Detail-loss kernel for TRN2 (8 NeuronCores), v2.

Reference computation (algebraically reduced):
  views = reshape(inputs, (98, 3, 256, 256)); d = infer - ref
  S[n] = sum_c d[n, c]                       (per-view 256x256 plane)
  loss = ( sum |S[n,h,w+1] - S[n,h,w-1]|     (zero-padded outside)
         + sum |S[n,h+1,w] - S[n,h-1,w]| ) / (4 * 98 * 258 * 256)

Sharding: 98 views padded to 104, 13 views per core (zero views add 0).

v2 changes vs v1 (55.7us):
  * inputs quantized to fp8e4m3 on host (rel err ~4.7e-4, gate is 2e-2):
    halves HBM traffic -> ~15us DMA floor per core.
  * host packs a/b rows interleaved per partition: DRAM x[p, v, c, 4rows]
    so each 2-view group loads with one DMA of 128 x 6KB descriptors,
    spread across the sync/scalar/gpsimd queues.
  * PE DoubleRow perf mode: lhsT = [I; -I] (fp8) contracts a-plane and
    b-plane in one matmul at 2x rate; 3 matmuls/view -> S in PSUM.
  * view-pair batching of S-copy / gw / abs ops (1024-elem units) to
    amortize per-instruction overheads; abs via tensor_scalar(abs_max)
    with accum_out on DVE, activation(Abs) accum on ACT, split by
    static assignment tables; a slice of gw subtracts on gpsimd.
Host: sum partials in float64, scale.
"""
import numpy as np
import ml_dtypes
import concourse.bass as bass
import concourse.mybir as mybir
from concourse import bacc
from concourse.tile import TileContext
from concourse.bass_utils import run_bass_kernel_spmd

N_CORES = 8
V = 13                       # views per core (98 -> 104 padded)
NPAIR = 7                    # view pairs (last is a singleton)
# per-view DMA queue: g=gpsimd (fast SWDGE ~200GB/s), s=sync, c=scalar
# (the two HWDGE queues share ~116GB/s total, so they get 2 views each,
# timed so pairs complete in order at a ~1.8us cadence)
DMA_QUEUE = "ggggsgcgsgcgg"
C, H, W = 3, 256, 256
SCALE = 1.0 / (4.0 * 98.0 * 258.0 * 256.0)
NCOL = 2 * NPAIR             # 7 gw cols + 7 gh cols

# engine assignment per pair index (tuned from trace)
COPY_ACT = {1, 2, 3, 4, 5}   # S-copy pairs routed to ACT (rest DVE)
GW_TT_GPSIMD = {1, 2, 3, 4}  # gw subtract pairs routed to gpsimd (rest DVE)
GW_ABS_ACT = {0, 2, 4}       # gw abs pairs on ACT (rest DVE reduce)
GH_ABS_ACT = {1, 3, 5}       # gh abs pairs on ACT (rest DVE reduce)

_cache = {}


def _weights():
    I = np.eye(128, dtype=np.float32)
    E = (np.eye(128) - np.eye(128, k=1)).astype(np.float32)   # out[p]=in[p]-in[p-1]
    O = (np.eye(128, k=-1) - np.eye(128)).astype(np.float32)  # out[p]=in[p+1]-in[p]
    wpair = np.stack([I, -I], axis=1)  # [128, 2, 128] DoubleRow stationary
    weo = np.stack([E, O], axis=1)     # [128, 2, 128]
    return wpair, weo


def _build():
    if "nc" in _cache:
        return _cache["nc"]
    f32 = mybir.dt.float32
    bf16 = mybir.dt.bfloat16
    f8 = mybir.dt.float8e4
    AluOp = mybir.AluOpType
    Act = mybir.ActivationFunctionType
    DR = mybir.MatmulPerfMode.DoubleRow

    nc = bacc.Bacc(None, target_bir_lowering=False)
    x = nc.declare_dram_parameter("x", [128, V, C, 2, 512], f8, isOutput=False)
    wp = nc.declare_dram_parameter("wp", [128, 2, 128], f8, isOutput=False)
    we = nc.declare_dram_parameter("we", [128, 2, 128], bf16, isOutput=False)
    y = nc.declare_dram_parameter("y", [128, NCOL], f32, isOutput=True)

    with TileContext(nc) as tc:
        with (
            tc.tile_pool(name="wpool", bufs=1) as wpool,
            tc.tile_pool(name="xp", bufs=V) as xpool,
            tc.tile_pool(name="sp", bufs=1) as spool,
            tc.tile_pool(name="gp", bufs=2) as gpool,
            tc.tile_pool(name="zp", bufs=2) as zpool,
            tc.tile_pool(name="cp", bufs=2) as cpool,
            tc.tile_pool(name="ap", bufs=1) as apool,
            tc.tile_pool(name="psS", bufs=2, space="PSUM") as psSp,
            tc.tile_pool(name="psG", bufs=2, space="PSUM") as psGp,
        ):
            # ---- weights then per-view input DMAs over 3 queues.
            # scalar-queue dma_starts issue early (before ACT compute
            # begins) so their ~0.67us seq cost is free.
            wpt = wpool.tile([128, 2, 128], f8)
            wet = wpool.tile([128, 2, 128], bf16)
            nc.sync.dma_start(out=wpt[:], in_=wp[:])
            nc.sync.dma_start(out=wet[:], in_=we[:])

            qmap = {"g": nc.gpsimd, "s": nc.sync, "c": nc.scalar}
            xts = []
            for v in range(V):
                xt = xpool.tile([128, C, 2, 512], f8, name=f"xv{v}", tag="xv")
                qmap[DMA_QUEUE[v]].dma_start(out=xt[:], in_=x[:, v])
                xts.append(xt)

            # persistent S pair-tiles: [pairview, s, w] with zero pad cols
            sts = [
                spool.tile([128, 2, 2, 258], bf16, name=f"st{i}") for i in range(4)
            ]
            for st in sts:
                nc.vector.memset(st[:, :, :, 0:1], 0.0)
                nc.vector.memset(st[:, :, :, 257:258], 0.0)

            acc = apool.tile([128, NCOL], f32)

            def npair_views(k):
                return 2 if k < NPAIR - 1 else 1

            def emit_gh(k):
                # gh via paired E/O matmuls on S + abs accum
                npv = npair_views(k)
                stp = sts[k % 4]
                psg = psGp.tile([128, 4, 256], f32, name="psg", tag="psg")
                nc.tensor.matmul(
                    psg[:, 0:npv, :], wet[:, 0, :],
                    stp[:, 0:npv, 1, 1:257], start=True, stop=True,
                )
                nc.tensor.matmul(
                    psg[:, npv : 2 * npv, :], wet[:, 1, :],
                    stp[:, 0:npv, 0, 1:257], start=True, stop=True,
                )
                hcol = acc[:, NPAIR + k : NPAIR + k + 1]
                if k in GH_ABS_ACT:
                    scg = cpool.tile([128, 4, 256], bf16, name="scg", tag="scg")
                    nc.scalar.activation(
                        scg[:, 0 : 2 * npv, :], psg[:, 0 : 2 * npv, :],
                        Act.Abs, accum_out=hcol,
                    )
                else:
                    nc.vector.tensor_reduce(
                        hcol, psg[:, 0 : 2 * npv, :], axis=mybir.AxisListType.XY,
                        op=AluOp.add, apply_absolute_value=True,
                    )

            for k in range(NPAIR):
                npv = npair_views(k)
                stp = sts[k % 4]
                # PE: S = sum_c (a_c - b_c) via 3 DoubleRow matmuls per view
                pss = psSp.tile([128, 2, 512], f32, name="pss", tag="pss")
                for vl in range(npv):
                    xt = xts[2 * k + vl]
                    for c in range(C):
                        nc.tensor.matmul(
                            pss[:, vl, :], wpt[:], xt[:, c],
                            start=(c == 0), stop=(c == C - 1), perf_mode=DR,
                        )
                # gh of the PREVIOUS pair is emitted here so its matmuls
                # sit behind this pair's channel matmuls in PE program
                # order, hiding the S-copy latency (software pipelining)
                if k >= 1:
                    emit_gh(k - 1)
                # S copy PSUM f32 -> SBUF bf16 (zero pad cols persist)
                sv = pss[:, 0:npv].rearrange("p v (s w) -> p v s w", s=2)
                sdst = stp[:, 0:npv, :, 1:257]
                if k in COPY_ACT:
                    nc.scalar.activation(sdst, sv, Act.Copy)
                else:
                    nc.vector.tensor_scalar_add(sdst, sv, 0.0)
                # gw = S[., w+1] - S[., w-1] (covers w edges via pad cols)
                gwt = gpool.tile([128, 2, 2, 256], bf16, name="gwt", tag="gwt")
                geng = nc.gpsimd if k in GW_TT_GPSIMD else nc.vector
                geng.tensor_tensor(
                    gwt[:, 0:npv], stp[:, 0:npv, :, 2:258],
                    stp[:, 0:npv, :, 0:256], AluOp.subtract,
                )
                wcol = acc[:, k : k + 1]
                if k in GW_ABS_ACT:
                    scr = zpool.tile([128, 2, 2, 256], bf16, name="scr", tag="scr")
                    nc.scalar.activation(
                        scr[:, 0:npv], gwt[:, 0:npv], Act.Abs, accum_out=wcol,
                    )
                else:
                    nc.vector.tensor_reduce(
                        wcol, gwt[:, 0:npv], axis=mybir.AxisListType.XYZ,
                        op=AluOp.add, apply_absolute_value=True,
                    )

            emit_gh(NPAIR - 1)

            nc.sync.dma_start(out=y[:], in_=acc[:])

    nc.finalize()
    _cache["nc"] = nc
    return nc


def _pack(infer, ref):
    """f32 [2,7,7,3,256,256] x2 -> per-core fp8 [128, V, C, 2, 512] packed."""
    f8 = ml_dtypes.float8_e4m3
    a = np.asarray(infer, dtype=np.float32).reshape(98, C, H, W).astype(f8)
    b = np.asarray(ref, dtype=np.float32).reshape(98, C, H, W).astype(f8)
    pad = np.zeros((6, C, H, W), f8)
    a = np.concatenate([a, pad], axis=0).reshape(104, C, 128, 2, W)
    b = np.concatenate([b, pad], axis=0).reshape(104, C, 128, 2, W)
    X = np.stack([a, b], axis=3)                # [104, C, 128, t, s, W]
    X = X.transpose(2, 0, 1, 3, 4, 5)           # [128, 104, C, t, s, W]
    cores = []
    for i in range(N_CORES):
        xi = np.ascontiguousarray(X[:, i * V : (i + 1) * V])
        cores.append(xi.reshape(128, V, C, 2, 512))
    return cores


def _run(infer, ref, trace=False, trace_kwargs=None):
    nc = _build()
    cores = _pack(infer, ref)
    wpair, weo = _weights()
    wpair = wpair.astype(ml_dtypes.float8_e4m3)
    weo = weo.astype(ml_dtypes.bfloat16)
    in_maps = [
        {"x": cores[i], "wp": wpair, "we": weo} for i in range(N_CORES)
    ]
    kwargs = {}
    if trace:
        kwargs["trace"] = True
        if trace_kwargs:
            kwargs["trace_kwargs"] = trace_kwargs
    out = run_bass_kernel_spmd(nc, in_maps, core_ids=list(range(N_CORES)), **kwargs)
    total = 0.0
    for res in out.results:
        total += res["y"].astype(np.float64).sum()
    loss = np.float32(total * SCALE)
    return loss, out


def kernel(infer, ref):
    loss, _ = _run(infer, ref)
    return np.asarray(loss, dtype=np.float32)
